# revision 10
# baseline (speedup 1.0000x reference)
"""Trainium2 Bass kernel for a Q4_0-quantized attention block (dense transformer).

Tensor-parallel over 8 NeuronCores: wq/wk/wv output features (heads) sharded
along the head dim (4 heads / 512 features per core), wo sharded along its
input dim; each core produces a partial [1024, 4096] output and the host sums
the 8 partials (the "all-reduce").

Precision: fp16 for x/wq/wk/wv and the rope'd q/k (softmax is sensitive to
absolute score error; fp16 keeps it ~3e-3), bf16 for exp tiles (fp16 would
overflow), v, attn weights and wo, f32 PSUM accumulation everywhere.
"""

import numpy as np
import ml_dtypes

DIM = 4096
SEQ = 1024
N_HEADS = 32
HD = 128          # head dim
GROUP = 64
NCORES = 8
OPC = DIM // NCORES      # output features per core (512) for q/k/v
HPC = OPC // HD          # heads per core (4)
KT = DIM // 128          # contraction tiles (32)
MT = SEQ // 128          # token tiles (8)

BF16 = ml_dtypes.bfloat16
F16 = np.float16

_STATE = {}


def _dequant(w8, s):
    """Q4_0 dequant, mirrors reference.dequantize_q40. Returns [DIM, DIM] f32
    indexed [a, b] where out[s, a] = sum_b x[s, b] * deq[a, b]."""
    w8 = np.asarray(w8)
    if w8.dtype != np.int8:
        w8 = w8.astype(np.int8)
    msb = w8 >> 4
    lsb = (w8 << 4) >> 4
    nib = np.concatenate([msb, lsb], axis=1).reshape(-1, GROUP)
    w = nib.astype(np.float32) * np.asarray(s, np.float32)[:, None]
    return w.reshape(DIM, DIM)


def _build_nc():
    import concourse.bacc as bacc
    import concourse.mybir as mybir
    from concourse import tile

    f32 = mybir.dt.float32
    f16 = mybir.dt.float16
    bf16 = mybir.dt.bfloat16
    Exp = mybir.ActivationFunctionType.Exp

    nc = bacc.Bacc("TRN2", target_bir_lowering=False, debug=False,
                   num_devices=NCORES)

    # --- DRAM parameters (per-core shard layouts, host-prepped) ---
    # All big tensors are partition-major so DMA runs are 8-64KB contiguous.
    xt_d = nc.declare_dram_parameter("xt", [128, KT * SEQ], f16, isOutput=False)
    wq_d = nc.declare_dram_parameter("wq", [8, 128, HPC * 512], f16, isOutput=False)
    wk_d = nc.declare_dram_parameter("wk", [HPC, 128, KT * 128], f16, isOutput=False)
    wv_d = nc.declare_dram_parameter("wv", [KT, 128, OPC], f16, isOutput=False)
    wo_d = nc.declare_dram_parameter("wo", [128, HPC * DIM], bf16, isOutput=False)
    cq_d = nc.declare_dram_parameter("cq", [128, SEQ], f16, isOutput=False)
    sq_d = nc.declare_dram_parameter("sq", [128, SEQ], f16, isOutput=False)
    ck_d = nc.declare_dram_parameter("ck", [128, SEQ], f16, isOutput=False)
    sk_d = nc.declare_dram_parameter("sk", [128, SEQ], f16, isOutput=False)
    tri_d = nc.declare_dram_parameter("tri", [128, 128], bf16, isOutput=False)
    one_d = nc.declare_dram_parameter("one", [128, 128], bf16, isOutput=False)
    out_d = nc.declare_dram_parameter("out", [SEQ, DIM], bf16, isOutput=True)

    with tile.TileContext(nc) as tc:
        with (
            tc.tile_pool(name="const", bufs=1) as constp,
            tc.tile_pool(name="qk", bufs=1) as qkp,
            tc.tile_pool(name="vsb", bufs=1) as vp,
            tc.tile_pool(name="attn", bufs=1) as attnp,
            tc.tile_pool(name="sums", bufs=2) as sumsp,
            tc.tile_pool(name="recip", bufs=2) as recipp,
        ):
            cq = constp.tile([128, SEQ], f16, tag="cq")
            sq = constp.tile([128, SEQ], f16, tag="sq")
            ck = constp.tile([128, SEQ], f16, tag="ck")
            sk = constp.tile([128, SEQ], f16, tag="sk")
            tri = constp.tile([128, 128], bf16, tag="tri")
            ones = constp.tile([128, 128], bf16, tag="one")
            q_emb = qkp.tile([128, HPC * SEQ], f16, tag="qemb")
            k_emb = qkp.tile([128, HPC * SEQ], f16, tag="kemb")
            v_sb = vp.tile([128, MT * OPC], bf16, tag="vsb")
            attn = attnp.tile([128, HPC * SEQ], bf16, tag="attn")

            # ============ Phases 1-2: projections (xt-resident) ============
            with (
                tc.tile_pool(name="xt", bufs=1) as xtp,
                tc.tile_pool(name="wvs", bufs=8) as wvsp,
            ):
                xt = xtp.tile([128, KT * SEQ], f16, tag="xt")
                # xt chunks are DMA'd inside the Q group loop below, in
                # consumption order (per-queue DMA is ~22GB/s; ordering and
                # queue spread set the startup ramp)

                # ---------- Q and K projections + RoPE ----------
                def rope(ps, emb, ot, ctile, stile, ropep):
                    # emb = q*c + swap_halves(q)*s (signs baked in s); the
                    # psum->sbuf copies split ACT/DVE so banks free fast
                    qbf = ropep.tile([128, SEQ], f16, tag="qbf", name="qbf")
                    tmp = ropep.tile([128, SEQ], f16, tag="tmp", name="tmp")
                    nc.vector.tensor_copy(qbf[:], ps[:])
                    nc.scalar.copy(tmp[0:64, :], ps[64:128, :])
                    nc.scalar.copy(tmp[64:128, :], ps[0:64, :])
                    dst = emb[:, ot * SEQ:(ot + 1) * SEQ]
                    nc.vector.tensor_mul(dst, qbf[:], ctile[:])
                    nc.vector.tensor_mul(tmp[:], tmp[:], stile[:])
                    nc.vector.tensor_add(dst, dst, tmp[:])

                wv_tiles = {}

                def wv_fetch(kt):
                    wvt = wvsp.tile([128, OPC], f16, tag="wvs", name=f"wvt{kt}")
                    (nc.sync if kt % 2 == 0 else nc.gpsimd).dma_start(
                        wvt[:], wv_d[kt])
                    wv_tiles[kt] = wvt

                with (
                    tc.tile_pool(name="slab", bufs=4) as slabp,
                    tc.tile_pool(name="strip", bufs=2) as stripp,
                    tc.tile_pool(name="rope", bufs=2) as ropep,
                    tc.tile_pool(name="psA", bufs=4, space="PSUM") as psA,
                ):
                    # Q: kt-group streamed (DMA-paced window right after
                    # start); all 4 head psums live so the PE has work the
                    # moment each xt/slab chunk lands
                    psq = [psA.tile([128, SEQ], f32, tag="psA", name=f"psq{ot}")
                           for ot in range(HPC)]
                    slabs = {}

                    def dma_group(g, fine):
                        slab = slabp.tile([128, HPC * 512], f16, tag="slab",
                                          name=f"slab{g}")
                        slabs[g] = slab
                        cw = 256 if fine else 512
                        for c in range(2048 // cw):
                            eng = nc.sync if c % 2 == 0 else nc.gpsimd
                            eng.dma_start(slab[:, c * cw:(c + 1) * cw],
                                          wq_d[g][:, c * cw:(c + 1) * cw])
                        xw = 256 if fine else 512
                        for c in range(4096 // xw):
                            lo = g * 4096 + c * xw
                            eng = nc.sync if c % 2 == 0 else nc.gpsimd
                            eng.dma_start(xt[:, lo:lo + xw], xt_d[:, lo:lo + xw])

                    dma_group(0, fine=True)
                    dma_group(1, fine=False)
                    for g in range(8):
                        slab = slabs[g]
                        if g == 0:
                            nc.sync.dma_start(cq[:], cq_d[:])
                            nc.gpsimd.dma_start(sq[:], sq_d[:])
                            nc.sync.dma_start(ck[:], ck_d[:])
                            nc.gpsimd.dma_start(sk[:], sk_d[:])
                            nc.sync.dma_start(tri[:], tri_d[:])
                            nc.gpsimd.dma_start(ones[:], one_d[:])
                        if g + 2 < 8:
                            dma_group(g + 2, fine=False)
                        for ot in range(HPC):
                            for ki in range(4):
                                kt = g * 4 + ki
                                for h2 in range(2):
                                    nc.tensor.matmul(
                                        psq[ot][:, h2 * 512:(h2 + 1) * 512],
                                        slab[:, ot * 512 + ki * 128: ot * 512 + (ki + 1) * 128],
                                        xt[:, kt * SEQ + h2 * 512: kt * SEQ + (h2 + 1) * 512],
                                        start=(kt == 0), stop=(kt == KT - 1),
                                    )
                    for ot in range(HPC):
                        rope(psq[ot], q_emb, ot, cq, sq, ropep)

                    for kt in range(8):
                        wv_fetch(kt)

                    # K: strip-based, xt already resident
                    for ot in range(HPC):
                        strip = stripp.tile([128, KT * 128], f16, tag="strip")
                        for g in range(8):
                            eng = nc.sync if g % 2 == 0 else nc.gpsimd
                            eng.dma_start(
                                strip[:, g * 4 * 128:(g + 1) * 4 * 128],
                                wk_d[ot][:, g * 4 * 128:(g + 1) * 4 * 128])
                        ps = psA.tile([128, SEQ], f32, tag="psA", name=f"psk{ot}")
                        for kt in range(KT):
                            for h2 in range(2):
                                nc.tensor.matmul(
                                    ps[:, h2 * 512:(h2 + 1) * 512],
                                    strip[:, kt * 128:(kt + 1) * 128],
                                    xt[:, kt * SEQ + h2 * 512: kt * SEQ + (h2 + 1) * 512],
                                    start=(kt == 0), stop=(kt == KT - 1),
                                )
                        rope(ps, k_emb, ot, ck, sk, ropep)

                # ------- V projection (kt-outer, all 8 PSUM banks) -------
                with tc.tile_pool(name="psV", bufs=8, space="PSUM") as psV:
                    psv = [psV.tile([128, OPC], f32, tag="psV", name=f"psV{mt}")
                           for mt in range(MT)]
                    for kt in range(KT):
                        if kt not in wv_tiles:
                            wv_fetch(kt)
                        wvt = wv_tiles[kt]
                        for mt in range(MT):
                            nc.tensor.matmul(
                                psv[mt][:],
                                xt[:, kt * SEQ + mt * 128: kt * SEQ + (mt + 1) * 128],
                                wvt[:],
                                start=(kt == 0), stop=(kt == KT - 1),
                            )
                    for mt in range(MT):
                        if mt % 2 == 0:
                            nc.scalar.copy(v_sb[:, mt * OPC:(mt + 1) * OPC], psv[mt][:])
                        else:
                            nc.vector.tensor_copy(v_sb[:, mt * OPC:(mt + 1) * OPC], psv[mt][:])

            # ============ Phases 3-4 (xt freed; wo resident) ============
            with tc.tile_pool(name="wo", bufs=1) as wop:
                wo_sb = wop.tile([128, HPC * DIM], bf16, tag="wo")
                for g in range(8):
                    lo, hi = g * 2048, (g + 1) * 2048
                    eng = nc.sync if g % 2 == 0 else nc.gpsimd
                    eng.dma_start(wo_sb[:, lo:hi], wo_d[:, lo:hi])

                # ---------- attention per head ----------
                # segment-granular: every scores psum is a single bank and
                # only diagonal segments wait on the tri mask, so a 2-segment
                # skew fully hides the scores->exp->(tri)->sum/av latency
                def seg_list():
                    out = []
                    for kt in range(MT):
                        lo = kt * 128
                        out.append((kt, lo, lo + 128, True))          # diag
                        if lo + 128 < 512:
                            out.append((kt, lo + 128, 512, False))    # mid
                        if max(lo + 128, 512) < SEQ:
                            out.append((kt, max(lo + 128, 512), SEQ, False))
                    return out

                with (
                    tc.tile_pool(name="exp", bufs=6) as expp,
                    tc.tile_pool(name="psS", bufs=4, space="PSUM") as psS,
                    tc.tile_pool(name="psM", bufs=1, space="PSUM") as psM,
                    tc.tile_pool(name="psO", bufs=1, space="PSUM") as psO,
                ):
                    for h in range(HPC):
                        pssum = psM.tile([128, SEQ], f32, tag="psM")
                        psout = psO.tile([128, SEQ], f32, tag="psO")
                        exp_tiles = {}

                        def emit_scores(seg, h=h):
                            kt, lo, hi, diag = seg
                            w = hi - lo
                            ps = psS.tile([128, 512], f32, tag="psS")
                            nc.tensor.matmul(
                                ps[:, :w],
                                k_emb[:, h * SEQ + kt * 128: h * SEQ + (kt + 1) * 128],
                                q_emb[:, h * SEQ + lo: h * SEQ + hi],
                                start=True, stop=True,
                            )
                            if kt not in exp_tiles:
                                exp_tiles[kt] = expp.tile(
                                    [128, SEQ], bf16, tag="exp", name=f"exp{kt}")
                            et = exp_tiles[kt]
                            nc.scalar.activation(et[:, lo:hi], ps[:, :w], Exp)
                            if diag:
                                # zero the below-diagonal half (gpsimd: keeps
                                # it out of the DVE/ACT FIFOs)
                                nc.gpsimd.tensor_mul(
                                    et[:, lo:hi], et[:, lo:hi], tri[:])

                        def emit_sum_av(seg, h=h):
                            kt, lo, hi, diag = seg
                            et = exp_tiles[kt]
                            last_kt = 3 if hi <= 512 else MT - 1
                            nc.tensor.matmul(
                                pssum[:, lo:hi], ones[:], et[:, lo:hi],
                                start=(kt == 0), stop=(kt == last_kt),
                            )
                            nc.tensor.matmul(
                                psout[:, lo:hi],
                                v_sb[:, kt * OPC + h * 128: kt * OPC + (h + 1) * 128],
                                et[:, lo:hi],
                                start=(kt == 0), stop=(kt == last_kt),
                            )

                        SKEW = 2
                        sl = seg_list()
                        for i, seg in enumerate(sl):
                            emit_scores(seg)
                            if i >= SKEW:
                                emit_sum_av(sl[i - SKEW])
                        for seg in sl[-SKEW:]:
                            emit_sum_av(seg)

                        # copy both psums to SBUF immediately (frees banks for
                        # the next head); recip+normalize run off-PSUM
                        sums_sb = sumsp.tile([128, SEQ], f32, tag="sums")
                        av_sb = sumsp.tile([128, SEQ], f32, tag="av")
                        nc.vector.tensor_copy(sums_sb[:], pssum[:])
                        nc.vector.tensor_copy(av_sb[:], psout[:])
                        rec = recipp.tile([128, SEQ], f32, tag="recip")
                        nc.vector.reciprocal_approx_fast(rec[:], sums_sb[:])
                        nc.vector.tensor_mul(
                            attn[:, h * SEQ:(h + 1) * SEQ], av_sb[:], rec[:])

                # ---------- output projection (partial) ----------
                with (
                    tc.tile_pool(name="outs", bufs=2) as outsp,
                    tc.tile_pool(name="psW", bufs=8, space="PSUM") as psW,
                ):
                    for mt in range(MT):
                        outt = outsp.tile([128, DIM], bf16, tag="outs")
                        pss = [psW.tile([128, 512], f32, tag="psW", name=f"psW{nt}")
                               for nt in range(8)]
                        for ktw in range(HPC):
                            for nt in range(8):
                                nc.tensor.matmul(
                                    pss[nt][:],
                                    attn[:, ktw * SEQ + mt * 128: ktw * SEQ + (mt + 1) * 128],
                                    wo_sb[:, ktw * DIM + nt * 512: ktw * DIM + (nt + 1) * 512],
                                    start=(ktw == 0), stop=(ktw == HPC - 1),
                                )
                        for nt in range(8):
                            if nt % 2 == 0:
                                nc.scalar.copy(outt[:, nt * 512:(nt + 1) * 512], pss[nt][:])
                            else:
                                nc.vector.tensor_copy(outt[:, nt * 512:(nt + 1) * 512], pss[nt][:])
                        for q in range(8):
                            eng = nc.sync if q % 2 == 0 else nc.gpsimd
                            eng.dma_start(
                                out_d[mt * 128:(mt + 1) * 128, q * 512:(q + 1) * 512],
                                outt[:, q * 512:(q + 1) * 512])

    nc.compile()
    return nc


def _get_nc():
    if "nc" not in _STATE:
        _STATE["nc"] = _build_nc()
    return _STATE["nc"]


def _part_major(m):
    """[KT*128, F] -> [128, KT*F]: partition-major repack so each SBUF
    partition's data is one contiguous DRAM run."""
    kt = m.shape[0] // 128
    return np.ascontiguousarray(
        m.reshape(kt, 128, -1).transpose(1, 0, 2)).reshape(128, -1)


def _prep_inputs(x, wq_w, wq_s, wk_w, wk_s, wv_w, wv_s, wo_w, wo_s, cos, sin):
    """Host-side shard prep. Returns in_maps (list of 8 dicts)."""
    xt = _part_major(np.asarray(x, np.float32)[0].T.astype(F16))

    dq = _dequant(wq_w, wq_s).T   # [b, a] = lhsT full
    dk = _dequant(wk_w, wk_s).T
    dv = _dequant(wv_w, wv_s).T
    do = _dequant(wo_w, wo_s).T

    cos = np.asarray(cos, np.float32)
    sin = np.asarray(sin, np.float32)
    cfull = np.concatenate([cos, cos], axis=1).T          # [128, SEQ]
    ssign = np.concatenate([-sin, sin], axis=1).T         # [128, SEQ]
    alpha = 1.0 / np.sqrt(np.float32(HD))
    cq = (cfull * alpha).astype(F16)
    sq = (ssign * alpha).astype(F16)
    ck = cfull.astype(F16)
    sk = ssign.astype(F16)
    tri = (np.arange(128)[:, None] <= np.arange(128)[None, :]).astype(BF16)
    one = np.ones((128, 128), BF16)

    in_maps = []
    for c in range(NCORES):
        sl = slice(c * OPC, (c + 1) * OPC)
        # strip: [4096, 512] -> [ot, c128, kt*128+o]
        def strip(m):
            t = m[:, sl].astype(F16).reshape(KT, 128, HPC, 128)
            return np.ascontiguousarray(t.transpose(2, 1, 0, 3)).reshape(
                HPC, 128, KT * 128)
        # slab: [4096, 512] -> [g8, c128, ot*512 + ki*128 + o]
        def slab(m):
            t = m[:, sl].astype(F16).reshape(8, 4, 128, HPC, 128)
            return np.ascontiguousarray(t.transpose(0, 2, 3, 1, 4)).reshape(
                8, 128, HPC * 512)
        in_maps.append({
            "xt": xt,
            "wq": slab(dq),
            "wk": strip(dk),
            "wv": np.ascontiguousarray(dv[:, sl].astype(F16)).reshape(KT, 128, OPC),
            "wo": _part_major(do[sl, :].astype(BF16)),
            "cq": cq, "sq": sq, "ck": ck, "sk": sk, "tri": tri, "one": one,
        })
    return in_maps


def kernel(x, wq_w, wq_s, wk_w, wk_s, wv_w, wv_s, wo_w, wo_s,
           cos, sin, cache_k, cache_v, mask, start_pos, _trace=False):
    from concourse.bass_utils import run_bass_kernel_spmd

    nc = _get_nc()
    in_maps = _prep_inputs(x, wq_w, wq_s, wk_w, wk_s, wv_w, wv_s,
                           wo_w, wo_s, cos, sin)
    res = run_bass_kernel_spmd(nc, in_maps, list(range(NCORES)), trace=_trace)
    _STATE["last_result"] = res
    out = np.zeros((SEQ, DIM), np.float64)
    for c in range(NCORES):
        out += res.results[c]["out"].astype(np.float64)
    return out.astype(np.float32).reshape(1, SEQ, DIM)


# revision 11
# speedup vs baseline: 1.0729x; 1.0729x over previous
"""Trainium2 Bass kernel for a Q4_0-quantized attention block (dense transformer).

Tensor-parallel over 8 NeuronCores: wq/wk/wv output features (heads) sharded
along the head dim (4 heads / 512 features per core), wo sharded along its
input dim; each core produces a partial [1024, 4096] output and the host sums
the 8 partials (the "all-reduce").

Precision: fp16 for x/wq/wk/wv and the rope'd q/k (softmax is sensitive to
absolute score error; fp16 keeps it ~3e-3), bf16 for exp tiles (fp16 would
overflow), v, attn weights and wo, f32 PSUM accumulation everywhere.
"""

import numpy as np
import ml_dtypes

DIM = 4096
SEQ = 1024
N_HEADS = 32
HD = 128          # head dim
GROUP = 64
NCORES = 8
OPC = DIM // NCORES      # output features per core (512) for q/k/v
HPC = OPC // HD          # heads per core (4)
KT = DIM // 128          # contraction tiles (32)
MT = SEQ // 128          # token tiles (8)

BF16 = ml_dtypes.bfloat16
F16 = np.float16

_STATE = {}


def _dequant(w8, s):
    """Q4_0 dequant, mirrors reference.dequantize_q40. Returns [DIM, DIM] f32
    indexed [a, b] where out[s, a] = sum_b x[s, b] * deq[a, b]."""
    w8 = np.asarray(w8)
    if w8.dtype != np.int8:
        w8 = w8.astype(np.int8)
    msb = w8 >> 4
    lsb = (w8 << 4) >> 4
    nib = np.concatenate([msb, lsb], axis=1).reshape(-1, GROUP)
    w = nib.astype(np.float32) * np.asarray(s, np.float32)[:, None]
    return w.reshape(DIM, DIM)


def _build_nc():
    import concourse.bacc as bacc
    import concourse.mybir as mybir
    from concourse import tile

    f32 = mybir.dt.float32
    f16 = mybir.dt.float16
    bf16 = mybir.dt.bfloat16
    Exp = mybir.ActivationFunctionType.Exp

    nc = bacc.Bacc("TRN2", target_bir_lowering=False, debug=False,
                   num_devices=NCORES)

    # --- DRAM parameters (per-core shard layouts, host-prepped) ---
    # All big tensors are partition-major so DMA runs are 8-64KB contiguous.
    xt_d = nc.declare_dram_parameter("xt", [128, KT * SEQ], f16, isOutput=False)
    wq_d = nc.declare_dram_parameter("wq", [8, 128, HPC * 512], f16, isOutput=False)
    wk_d = nc.declare_dram_parameter("wk", [HPC, 128, KT * 128], f16, isOutput=False)
    wv_d = nc.declare_dram_parameter("wv", [KT, 128, OPC], f16, isOutput=False)
    wo_d = nc.declare_dram_parameter("wo", [128, HPC * DIM], bf16, isOutput=False)
    cq_d = nc.declare_dram_parameter("cq", [128, SEQ], f16, isOutput=False)
    sq_d = nc.declare_dram_parameter("sq", [128, SEQ], f16, isOutput=False)
    ck_d = nc.declare_dram_parameter("ck", [128, SEQ], f16, isOutput=False)
    sk_d = nc.declare_dram_parameter("sk", [128, SEQ], f16, isOutput=False)
    tri_d = nc.declare_dram_parameter("tri", [128, 128], bf16, isOutput=False)
    one_d = nc.declare_dram_parameter("one", [128, 128], bf16, isOutput=False)
    out_d = nc.declare_dram_parameter("out", [SEQ, DIM], bf16, isOutput=True)

    with tile.TileContext(nc) as tc:
        with (
            tc.tile_pool(name="const", bufs=1) as constp,
            tc.tile_pool(name="qk", bufs=1) as qkp,
            tc.tile_pool(name="vsb", bufs=1) as vp,
            tc.tile_pool(name="attn", bufs=1) as attnp,
            tc.tile_pool(name="sums", bufs=2) as sumsp,
            tc.tile_pool(name="recip", bufs=2) as recipp,
        ):
            cq = constp.tile([128, SEQ], f16, tag="cq")
            sq = constp.tile([128, SEQ], f16, tag="sq")
            ck = constp.tile([128, SEQ], f16, tag="ck")
            sk = constp.tile([128, SEQ], f16, tag="sk")
            tri = constp.tile([128, 128], bf16, tag="tri")
            ones = constp.tile([128, 128], bf16, tag="one")
            q_emb = qkp.tile([128, HPC * SEQ], f16, tag="qemb")
            k_emb = qkp.tile([128, HPC * SEQ], f16, tag="kemb")
            v_sb = vp.tile([128, MT * OPC], bf16, tag="vsb")
            attn = attnp.tile([128, HPC * SEQ], bf16, tag="attn")

            # ============ Phases 1-2: projections (xt-resident) ============
            with (
                tc.tile_pool(name="xt", bufs=1) as xtp,
                tc.tile_pool(name="wvs", bufs=4) as wvsp,
            ):
                xt = xtp.tile([128, KT * SEQ], f16, tag="xt")
                # xt chunks are DMA'd inside the Q group loop below, in
                # consumption order (per-queue DMA is ~22GB/s; ordering and
                # queue spread set the startup ramp)

                # ---------- Q and K projections + RoPE ----------
                def rope(ps, emb, ot, ctile, stile, ropep):
                    # emb = q*c + swap_halves(q)*s (signs baked in s); the
                    # psum->sbuf copies split ACT/DVE so banks free fast
                    qbf = ropep.tile([128, SEQ], f16, tag="qbf", name="qbf")
                    tmp = ropep.tile([128, SEQ], f16, tag="tmp", name="tmp")
                    nc.vector.tensor_copy(qbf[:], ps[:])
                    nc.scalar.copy(tmp[0:64, :], ps[64:128, :])
                    nc.scalar.copy(tmp[64:128, :], ps[0:64, :])
                    dst = emb[:, ot * SEQ:(ot + 1) * SEQ]
                    nc.vector.tensor_mul(dst, qbf[:], ctile[:])
                    nc.vector.tensor_mul(tmp[:], tmp[:], stile[:])
                    nc.vector.tensor_add(dst, dst, tmp[:])

                wv_tiles = {}

                def wv_fetch(kt):
                    wvt = wvsp.tile([128, OPC], f16, tag="wvs", name=f"wvt{kt}")
                    (nc.sync if kt % 2 == 0 else nc.gpsimd).dma_start(
                        wvt[:], wv_d[kt])
                    wv_tiles[kt] = wvt

                with (
                    tc.tile_pool(name="slab", bufs=8) as slabp,
                    tc.tile_pool(name="strip", bufs=2) as stripp,
                    tc.tile_pool(name="rope", bufs=2) as ropep,
                    tc.tile_pool(name="psA", bufs=4, space="PSUM") as psA,
                ):
                    # Q: kt-group streamed (DMA-paced window right after
                    # start); all 4 head psums live so the PE has work the
                    # moment each xt/slab chunk lands
                    psq = [psA.tile([128, SEQ], f32, tag="psA", name=f"psq{ot}")
                           for ot in range(HPC)]
                    slabs = {}

                    def dma_group(g, fine):
                        slab = slabp.tile([128, HPC * 512], f16, tag="slab",
                                          name=f"slab{g}")
                        slabs[g] = slab
                        cw = 256 if fine else 512
                        for c in range(2048 // cw):
                            eng = nc.sync if c % 2 == 0 else nc.gpsimd
                            eng.dma_start(slab[:, c * cw:(c + 1) * cw],
                                          wq_d[g][:, c * cw:(c + 1) * cw])
                        xw = 256 if fine else 512
                        for c in range(4096 // xw):
                            lo = g * 4096 + c * xw
                            eng = nc.sync if c % 2 == 0 else nc.gpsimd
                            eng.dma_start(xt[:, lo:lo + xw], xt_d[:, lo:lo + xw])

                    dma_group(0, fine=True)
                    dma_group(1, fine=False)
                    for g in range(8):
                        slab = slabs[g]
                        if g == 0:
                            nc.sync.dma_start(cq[:], cq_d[:])
                            nc.gpsimd.dma_start(sq[:], sq_d[:])
                            nc.sync.dma_start(ck[:], ck_d[:])
                            nc.gpsimd.dma_start(sk[:], sk_d[:])
                            nc.sync.dma_start(tri[:], tri_d[:])
                            nc.gpsimd.dma_start(ones[:], one_d[:])
                        if g + 2 < 8:
                            dma_group(g + 2, fine=False)
                        for ot in range(HPC):
                            for ki in range(4):
                                kt = g * 4 + ki
                                for h2 in range(2):
                                    nc.tensor.matmul(
                                        psq[ot][:, h2 * 512:(h2 + 1) * 512],
                                        slab[:, ot * 512 + ki * 128: ot * 512 + (ki + 1) * 128],
                                        xt[:, kt * SEQ + h2 * 512: kt * SEQ + (h2 + 1) * 512],
                                        start=(kt == 0), stop=(kt == KT - 1),
                                    )
                    for ot in range(HPC):
                        rope(psq[ot], q_emb, ot, cq, sq, ropep)

                    for kt in range(4):
                        wv_fetch(kt)

                    # K: strip-based, xt already resident
                    for ot in range(HPC):
                        strip = stripp.tile([128, KT * 128], f16, tag="strip")
                        for g in range(8):
                            eng = nc.sync if g % 2 == 0 else nc.gpsimd
                            eng.dma_start(
                                strip[:, g * 4 * 128:(g + 1) * 4 * 128],
                                wk_d[ot][:, g * 4 * 128:(g + 1) * 4 * 128])
                        ps = psA.tile([128, SEQ], f32, tag="psA", name=f"psk{ot}")
                        for kt in range(KT):
                            for h2 in range(2):
                                nc.tensor.matmul(
                                    ps[:, h2 * 512:(h2 + 1) * 512],
                                    strip[:, kt * 128:(kt + 1) * 128],
                                    xt[:, kt * SEQ + h2 * 512: kt * SEQ + (h2 + 1) * 512],
                                    start=(kt == 0), stop=(kt == KT - 1),
                                )
                        rope(ps, k_emb, ot, ck, sk, ropep)

                # ------- V projection (kt-outer, all 8 PSUM banks) -------
                with tc.tile_pool(name="psV", bufs=8, space="PSUM") as psV:
                    psv = [psV.tile([128, OPC], f32, tag="psV", name=f"psV{mt}")
                           for mt in range(MT)]
                    for kt in range(KT):
                        if kt not in wv_tiles:
                            wv_fetch(kt)
                        wvt = wv_tiles[kt]
                        for mt in range(MT):
                            nc.tensor.matmul(
                                psv[mt][:],
                                xt[:, kt * SEQ + mt * 128: kt * SEQ + (mt + 1) * 128],
                                wvt[:],
                                start=(kt == 0), stop=(kt == KT - 1),
                            )
                    for mt in range(MT):
                        if mt % 2 == 0:
                            nc.scalar.copy(v_sb[:, mt * OPC:(mt + 1) * OPC], psv[mt][:])
                        else:
                            nc.vector.tensor_copy(v_sb[:, mt * OPC:(mt + 1) * OPC], psv[mt][:])

            # ============ Phases 3-4 (xt freed; wo resident) ============
            with tc.tile_pool(name="wo", bufs=1) as wop:
                wo_sb = wop.tile([128, HPC * DIM], bf16, tag="wo")
                for g in range(8):
                    lo, hi = g * 2048, (g + 1) * 2048
                    nc.sync.dma_start(wo_sb[:, lo:hi], wo_d[:, lo:hi])

                # ---------- attention per head ----------
                # bank-split segments: each scores psum is a single bank,
                # so 4 psS slots give a 3-segment skew that hides the
                # scores->exp->(tri)->sum/av dependency latency
                def seg_list():
                    out = []
                    for kt in range(MT):
                        lo = kt * 128
                        if lo < 512:
                            out.append((kt, lo, 512, True))
                            out.append((kt, 512, SEQ, False))
                        else:
                            out.append((kt, lo, SEQ, True))
                    return out

                with (
                    tc.tile_pool(name="exp", bufs=6) as expp,
                    tc.tile_pool(name="psS", bufs=4, space="PSUM") as psS,
                    tc.tile_pool(name="psM", bufs=1, space="PSUM") as psM,
                    tc.tile_pool(name="psO", bufs=1, space="PSUM") as psO,
                ):
                    for h in range(HPC):
                        pssum = psM.tile([128, SEQ], f32, tag="psM")
                        psout = psO.tile([128, SEQ], f32, tag="psO")
                        exp_tiles = {}

                        def emit_scores(seg, h=h):
                            kt, lo, hi, diag = seg
                            w = hi - lo
                            ps = psS.tile([128, 512], f32, tag="psS")
                            nc.tensor.matmul(
                                ps[:, :w],
                                k_emb[:, h * SEQ + kt * 128: h * SEQ + (kt + 1) * 128],
                                q_emb[:, h * SEQ + lo: h * SEQ + hi],
                                start=True, stop=True,
                            )
                            if kt not in exp_tiles:
                                exp_tiles[kt] = expp.tile(
                                    [128, SEQ], bf16, tag="exp", name=f"exp{kt}")
                            et = exp_tiles[kt]
                            nc.scalar.activation(et[:, lo:hi], ps[:, :w], Exp)
                            if diag:
                                # zero the below-diagonal half of the diag
                                # block (gpsimd: keeps it out of the DVE/ACT
                                # FIFOs)
                                dlo = kt * 128
                                nc.gpsimd.tensor_mul(
                                    et[:, dlo:dlo + 128], et[:, dlo:dlo + 128],
                                    tri[:])

                        def emit_sum_av(seg, h=h):
                            kt, lo, hi, diag = seg
                            et = exp_tiles[kt]
                            last_kt = 3 if hi <= 512 else MT - 1
                            nc.tensor.matmul(
                                pssum[:, lo:hi], ones[:], et[:, lo:hi],
                                start=(kt == 0), stop=(kt == last_kt),
                            )
                            nc.tensor.matmul(
                                psout[:, lo:hi],
                                v_sb[:, kt * OPC + h * 128: kt * OPC + (h + 1) * 128],
                                et[:, lo:hi],
                                start=(kt == 0), stop=(kt == last_kt),
                            )

                        SKEW = 3
                        sl = seg_list()
                        for i, seg in enumerate(sl):
                            emit_scores(seg)
                            if i >= SKEW:
                                emit_sum_av(sl[i - SKEW])
                        for seg in sl[-SKEW:]:
                            emit_sum_av(seg)

                        # copy both psums to SBUF immediately (frees banks for
                        # the next head); recip+normalize run off-PSUM
                        sums_sb = sumsp.tile([128, SEQ], f32, tag="sums")
                        av_sb = sumsp.tile([128, SEQ], f32, tag="av")
                        nc.vector.tensor_copy(sums_sb[:], pssum[:])
                        nc.vector.tensor_copy(av_sb[:], psout[:])
                        rec = recipp.tile([128, SEQ], f32, tag="recip")
                        nc.vector.reciprocal_approx_fast(rec[:], sums_sb[:])
                        nc.vector.tensor_mul(
                            attn[:, h * SEQ:(h + 1) * SEQ], av_sb[:], rec[:])

                # ---------- output projection (partial) ----------
                with (
                    tc.tile_pool(name="outs", bufs=2) as outsp,
                    tc.tile_pool(name="psW", bufs=8, space="PSUM") as psW,
                ):
                    for mt in range(MT):
                        outt = outsp.tile([128, DIM], bf16, tag="outs")
                        pss = [psW.tile([128, 512], f32, tag="psW", name=f"psW{nt}")
                               for nt in range(8)]
                        for ktw in range(HPC):
                            for nt in range(8):
                                nc.tensor.matmul(
                                    pss[nt][:],
                                    attn[:, ktw * SEQ + mt * 128: ktw * SEQ + (mt + 1) * 128],
                                    wo_sb[:, ktw * DIM + nt * 512: ktw * DIM + (nt + 1) * 512],
                                    start=(ktw == 0), stop=(ktw == HPC - 1),
                                )
                        for nt in range(8):
                            if nt % 2 == 0:
                                nc.scalar.copy(outt[:, nt * 512:(nt + 1) * 512], pss[nt][:])
                            else:
                                nc.vector.tensor_copy(outt[:, nt * 512:(nt + 1) * 512], pss[nt][:])
                        for q in range(8):
                            eng = nc.sync if q % 2 == 0 else nc.gpsimd
                            eng.dma_start(
                                out_d[mt * 128:(mt + 1) * 128, q * 512:(q + 1) * 512],
                                outt[:, q * 512:(q + 1) * 512])

    nc.compile()
    return nc


def _get_nc():
    if "nc" not in _STATE:
        _STATE["nc"] = _build_nc()
    return _STATE["nc"]


def _part_major(m):
    """[KT*128, F] -> [128, KT*F]: partition-major repack so each SBUF
    partition's data is one contiguous DRAM run."""
    kt = m.shape[0] // 128
    return np.ascontiguousarray(
        m.reshape(kt, 128, -1).transpose(1, 0, 2)).reshape(128, -1)


def _prep_inputs(x, wq_w, wq_s, wk_w, wk_s, wv_w, wv_s, wo_w, wo_s, cos, sin):
    """Host-side shard prep. Returns in_maps (list of 8 dicts)."""
    xt = _part_major(np.asarray(x, np.float32)[0].T.astype(F16))

    dq = _dequant(wq_w, wq_s).T   # [b, a] = lhsT full
    dk = _dequant(wk_w, wk_s).T
    dv = _dequant(wv_w, wv_s).T
    do = _dequant(wo_w, wo_s).T

    cos = np.asarray(cos, np.float32)
    sin = np.asarray(sin, np.float32)
    cfull = np.concatenate([cos, cos], axis=1).T          # [128, SEQ]
    ssign = np.concatenate([-sin, sin], axis=1).T         # [128, SEQ]
    alpha = 1.0 / np.sqrt(np.float32(HD))
    cq = (cfull * alpha).astype(F16)
    sq = (ssign * alpha).astype(F16)
    ck = cfull.astype(F16)
    sk = ssign.astype(F16)
    tri = (np.arange(128)[:, None] <= np.arange(128)[None, :]).astype(BF16)
    one = np.ones((128, 128), BF16)

    in_maps = []
    for c in range(NCORES):
        sl = slice(c * OPC, (c + 1) * OPC)
        # strip: [4096, 512] -> [ot, c128, kt*128+o]
        def strip(m):
            t = m[:, sl].astype(F16).reshape(KT, 128, HPC, 128)
            return np.ascontiguousarray(t.transpose(2, 1, 0, 3)).reshape(
                HPC, 128, KT * 128)
        # slab: [4096, 512] -> [g8, c128, ot*512 + ki*128 + o]
        def slab(m):
            t = m[:, sl].astype(F16).reshape(8, 4, 128, HPC, 128)
            return np.ascontiguousarray(t.transpose(0, 2, 3, 1, 4)).reshape(
                8, 128, HPC * 512)
        in_maps.append({
            "xt": xt,
            "wq": slab(dq),
            "wk": strip(dk),
            "wv": np.ascontiguousarray(dv[:, sl].astype(F16)).reshape(KT, 128, OPC),
            "wo": _part_major(do[sl, :].astype(BF16)),
            "cq": cq, "sq": sq, "ck": ck, "sk": sk, "tri": tri, "one": one,
        })
    return in_maps


def kernel(x, wq_w, wq_s, wk_w, wk_s, wv_w, wv_s, wo_w, wo_s,
           cos, sin, cache_k, cache_v, mask, start_pos, _trace=False):
    from concourse.bass_utils import run_bass_kernel_spmd

    nc = _get_nc()
    in_maps = _prep_inputs(x, wq_w, wq_s, wk_w, wk_s, wv_w, wv_s,
                           wo_w, wo_s, cos, sin)
    res = run_bass_kernel_spmd(nc, in_maps, list(range(NCORES)), trace=_trace)
    _STATE["last_result"] = res
    out = np.zeros((SEQ, DIM), np.float64)
    for c in range(NCORES):
        out += res.results[c]["out"].astype(np.float64)
    return out.astype(np.float32).reshape(1, SEQ, DIM)


# revision 13
# speedup vs baseline: 1.0869x; 1.0130x over previous
"""Trainium2 Bass kernel for a Q4_0-quantized attention block (dense transformer).

Tensor-parallel over 8 NeuronCores: wq/wk/wv output features (heads) sharded
along the head dim (4 heads / 512 features per core), wo sharded along its
input dim; each core produces a partial [1024, 4096] output and the host sums
the 8 partials (the "all-reduce").

Precision: fp16 for x/wq/wk/wv and the rope'd q/k (softmax is sensitive to
absolute score error; fp16 keeps it ~3e-3), bf16 for exp tiles (fp16 would
overflow), v, attn weights and wo, f32 PSUM accumulation everywhere.
"""

import numpy as np
import ml_dtypes

DIM = 4096
SEQ = 1024
N_HEADS = 32
HD = 128          # head dim
GROUP = 64
NCORES = 8
OPC = DIM // NCORES      # output features per core (512) for q/k/v
HPC = OPC // HD          # heads per core (4)
KT = DIM // 128          # contraction tiles (32)
MT = SEQ // 128          # token tiles (8)

BF16 = ml_dtypes.bfloat16
F16 = np.float16

_STATE = {}


def _dequant(w8, s):
    """Q4_0 dequant, mirrors reference.dequantize_q40. Returns [DIM, DIM] f32
    indexed [a, b] where out[s, a] = sum_b x[s, b] * deq[a, b]."""
    w8 = np.asarray(w8)
    if w8.dtype != np.int8:
        w8 = w8.astype(np.int8)
    msb = w8 >> 4
    lsb = (w8 << 4) >> 4
    nib = np.concatenate([msb, lsb], axis=1).reshape(-1, GROUP)
    w = nib.astype(np.float32) * np.asarray(s, np.float32)[:, None]
    return w.reshape(DIM, DIM)


def _build_nc():
    import concourse.bacc as bacc
    import concourse.mybir as mybir
    from concourse import tile

    f32 = mybir.dt.float32
    f16 = mybir.dt.float16
    bf16 = mybir.dt.bfloat16
    Exp = mybir.ActivationFunctionType.Exp

    nc = bacc.Bacc("TRN2", target_bir_lowering=False, debug=False,
                   num_devices=NCORES)

    # --- DRAM parameters (per-core shard layouts, host-prepped) ---
    # All big tensors are partition-major so DMA runs are 8-64KB contiguous.
    xt_d = nc.declare_dram_parameter("xt", [128, KT * SEQ], f16, isOutput=False)
    wq_d = nc.declare_dram_parameter("wq", [8, 128, HPC * 512], f16, isOutput=False)
    wk_d = nc.declare_dram_parameter("wk", [HPC, 128, KT * 128], f16, isOutput=False)
    wv_d = nc.declare_dram_parameter("wv", [KT, 128, OPC], f16, isOutput=False)
    wo_d = nc.declare_dram_parameter("wo", [128, HPC * DIM], bf16, isOutput=False)
    cq_d = nc.declare_dram_parameter("cq", [128, SEQ], f16, isOutput=False)
    sq_d = nc.declare_dram_parameter("sq", [128, SEQ], f16, isOutput=False)
    ck_d = nc.declare_dram_parameter("ck", [128, SEQ], f16, isOutput=False)
    sk_d = nc.declare_dram_parameter("sk", [128, SEQ], f16, isOutput=False)
    tri_d = nc.declare_dram_parameter("tri", [128, 128], bf16, isOutput=False)
    one_d = nc.declare_dram_parameter("one", [128, 128], bf16, isOutput=False)
    out_d = nc.declare_dram_parameter("out", [SEQ, DIM], bf16, isOutput=True)

    with tile.TileContext(nc) as tc:
        with (
            tc.tile_pool(name="const", bufs=1) as constp,
            tc.tile_pool(name="qk", bufs=1) as qkp,
            tc.tile_pool(name="vsb", bufs=1) as vp,
            tc.tile_pool(name="attn", bufs=1) as attnp,
            tc.tile_pool(name="sums", bufs=2) as sumsp,
            tc.tile_pool(name="recip", bufs=2) as recipp,
        ):
            cq = constp.tile([128, SEQ], f16, tag="cq")
            sq = constp.tile([128, SEQ], f16, tag="sq")
            ck = constp.tile([128, SEQ], f16, tag="ck")
            sk = constp.tile([128, SEQ], f16, tag="sk")
            tri = constp.tile([128, 128], bf16, tag="tri")
            ones = constp.tile([128, 128], bf16, tag="one")
            q_emb = qkp.tile([128, HPC * SEQ], f16, tag="qemb")
            k_emb = qkp.tile([128, HPC * SEQ], f16, tag="kemb")
            v_sb = vp.tile([128, MT * OPC], bf16, tag="vsb")
            attn = attnp.tile([128, HPC * SEQ], bf16, tag="attn")

            # ============ Phases 1-2: projections (xt-resident) ============
            with (
                tc.tile_pool(name="xt", bufs=1) as xtp,
                tc.tile_pool(name="wvs", bufs=4) as wvsp,
            ):
                xt = xtp.tile([128, KT * SEQ], f16, tag="xt")
                # xt chunks are DMA'd inside the Q group loop below, in
                # consumption order (per-queue DMA is ~22GB/s; ordering and
                # queue spread set the startup ramp)

                # ---------- Q and K projections + RoPE ----------
                def rope(ps, emb, ot, ctile, stile, ropep):
                    # emb = q*c + swap_halves(q)*s (signs baked in s); the
                    # psum->sbuf copies split ACT/DVE so banks free fast
                    qbf = ropep.tile([128, SEQ], f16, tag="qbf", name="qbf")
                    tmp = ropep.tile([128, SEQ], f16, tag="tmp", name="tmp")
                    nc.vector.tensor_copy(qbf[:], ps[:])
                    nc.scalar.copy(tmp[0:64, :], ps[64:128, :])
                    nc.scalar.copy(tmp[64:128, :], ps[0:64, :])
                    dst = emb[:, ot * SEQ:(ot + 1) * SEQ]
                    nc.vector.tensor_mul(dst, qbf[:], ctile[:])
                    nc.vector.tensor_mul(tmp[:], tmp[:], stile[:])
                    nc.vector.tensor_add(dst, dst, tmp[:])

                wv_tiles = {}

                def wv_fetch(kt):
                    wvt = wvsp.tile([128, OPC], f16, tag="wvs", name=f"wvt{kt}")
                    (nc.sync if kt % 2 == 0 else nc.gpsimd).dma_start(
                        wvt[:], wv_d[kt])
                    wv_tiles[kt] = wvt

                psA_cm = tc.tile_pool(name="psA", bufs=4, space="PSUM")
                psA = psA_cm.__enter__()
                with (
                    tc.tile_pool(name="slab", bufs=8) as slabp,
                    tc.tile_pool(name="strip", bufs=2) as stripp,
                    tc.tile_pool(name="rope", bufs=2) as ropep,
                ):
                    # Q: kt-group streamed (DMA-paced window right after
                    # start); all 4 head psums live so the PE has work the
                    # moment each xt/slab chunk lands
                    psq = [psA.tile([128, SEQ], f32, tag="psA", name=f"psq{ot}")
                           for ot in range(HPC)]
                    slabs = {}

                    def dma_group(g, fine):
                        slab = slabp.tile([128, HPC * 512], f16, tag="slab",
                                          name=f"slab{g}")
                        slabs[g] = slab
                        cw = 256 if fine else 512
                        for c in range(2048 // cw):
                            eng = nc.sync if c % 2 == 0 else nc.gpsimd
                            eng.dma_start(slab[:, c * cw:(c + 1) * cw],
                                          wq_d[g][:, c * cw:(c + 1) * cw])
                        xw = 256 if fine else 512
                        for c in range(4096 // xw):
                            lo = g * 4096 + c * xw
                            eng = nc.sync if c % 2 == 0 else nc.gpsimd
                            eng.dma_start(xt[:, lo:lo + xw], xt_d[:, lo:lo + xw])

                    dma_group(0, fine=True)
                    dma_group(1, fine=False)
                    for g in range(8):
                        slab = slabs[g]
                        if g == 0:
                            nc.sync.dma_start(cq[:], cq_d[:])
                            nc.gpsimd.dma_start(sq[:], sq_d[:])
                            nc.sync.dma_start(ck[:], ck_d[:])
                            nc.gpsimd.dma_start(sk[:], sk_d[:])
                            nc.sync.dma_start(tri[:], tri_d[:])
                            nc.gpsimd.dma_start(ones[:], one_d[:])
                        if g + 2 < 8:
                            dma_group(g + 2, fine=False)
                        for ot in range(HPC):
                            for ki in range(4):
                                kt = g * 4 + ki
                                for h2 in range(2):
                                    nc.tensor.matmul(
                                        psq[ot][:, h2 * 512:(h2 + 1) * 512],
                                        slab[:, ot * 512 + ki * 128: ot * 512 + (ki + 1) * 128],
                                        xt[:, kt * SEQ + h2 * 512: kt * SEQ + (h2 + 1) * 512],
                                        start=(kt == 0), stop=(kt == KT - 1),
                                    )
                    for ot in range(HPC):
                        rope(psq[ot], q_emb, ot, cq, sq, ropep)

                    for kt in range(4):
                        wv_fetch(kt)

                    # K: strip-based, xt already resident
                    for ot in range(HPC):
                        strip = stripp.tile([128, KT * 128], f16, tag="strip")
                        for g in range(8):
                            eng = nc.sync if g % 2 == 0 else nc.gpsimd
                            eng.dma_start(
                                strip[:, g * 4 * 128:(g + 1) * 4 * 128],
                                wk_d[ot][:, g * 4 * 128:(g + 1) * 4 * 128])
                        ps = psA.tile([128, SEQ], f32, tag="psA", name=f"psk{ot}")
                        for kt in range(KT):
                            for h2 in range(2):
                                nc.tensor.matmul(
                                    ps[:, h2 * 512:(h2 + 1) * 512],
                                    strip[:, kt * 128:(kt + 1) * 128],
                                    xt[:, kt * SEQ + h2 * 512: kt * SEQ + (h2 + 1) * 512],
                                    start=(kt == 0), stop=(kt == KT - 1),
                                )
                        rope(ps, k_emb, ot, ck, sk, ropep)

                # ------- V projection (kt-outer, all 8 PSUM banks) -------
                # psums come from the psA pool (same slot size) so there is
                # no pool-open barrier between K's rope drain and V's start
                psv = [psA.tile([128, SEQ], f32, tag="psA", name=f"psV{m2}")
                       for m2 in range(4)]
                for kt in range(KT):
                    if kt not in wv_tiles:
                        wv_fetch(kt)
                    wvt = wv_tiles[kt]
                    for mt in range(MT):
                        nc.tensor.matmul(
                            psv[mt // 2][:, (mt % 2) * OPC:(mt % 2 + 1) * OPC],
                            xt[:, kt * SEQ + mt * 128: kt * SEQ + (mt + 1) * 128],
                            wvt[:],
                            start=(kt == 0), stop=(kt == KT - 1),
                        )
                for m2 in range(4):
                    if m2 % 2 == 0:
                        nc.scalar.copy(v_sb[:, m2 * 2 * OPC:(m2 + 1) * 2 * OPC], psv[m2][:])
                    else:
                        nc.vector.tensor_copy(v_sb[:, m2 * 2 * OPC:(m2 + 1) * 2 * OPC], psv[m2][:])
                psA_cm.__exit__(None, None, None)

            # ============ Phases 3-4 (xt freed; wo resident) ============
            with tc.tile_pool(name="wo", bufs=1) as wop:
                wo_sb = wop.tile([128, HPC * DIM], bf16, tag="wo")
                for g in range(8):
                    lo, hi = g * 2048, (g + 1) * 2048
                    nc.sync.dma_start(wo_sb[:, lo:hi], wo_d[:, lo:hi])

                # ---------- attention per head ----------
                # bank-split segments: each scores psum is a single bank,
                # so 4 psS slots give a 3-segment skew that hides the
                # scores->exp->(tri)->sum/av dependency latency
                def seg_list():
                    out = []
                    for kt in range(MT):
                        lo = kt * 128
                        if lo < 512:
                            out.append((kt, lo, 512, True))
                            out.append((kt, 512, SEQ, False))
                        else:
                            out.append((kt, lo, SEQ, True))
                    return out

                with (
                    tc.tile_pool(name="exp", bufs=12) as expp,
                    tc.tile_pool(name="psS", bufs=4, space="PSUM") as psS,
                    tc.tile_pool(name="psM", bufs=1, space="PSUM") as psM,
                    tc.tile_pool(name="psO", bufs=1, space="PSUM") as psO,
                ):
                    for h in range(HPC):
                        pssum = psM.tile([128, SEQ], f32, tag="psM")
                        psout = psO.tile([128, SEQ], f32, tag="psO")
                        exp_tiles = {}

                        def emit_scores(seg, h=h):
                            kt, lo, hi, diag = seg
                            w = hi - lo
                            ps = psS.tile([128, 512], f32, tag="psS")
                            nc.tensor.matmul(
                                ps[:, :w],
                                k_emb[:, h * SEQ + kt * 128: h * SEQ + (kt + 1) * 128],
                                q_emb[:, h * SEQ + lo: h * SEQ + hi],
                                start=True, stop=True,
                            )
                            if kt not in exp_tiles:
                                exp_tiles[kt] = expp.tile(
                                    [128, SEQ], bf16, tag="exp", name=f"exp{kt}")
                            et = exp_tiles[kt]
                            nc.scalar.activation(et[:, lo:hi], ps[:, :w], Exp)
                            if diag:
                                # zero the below-diagonal half of the diag
                                # block (gpsimd: keeps it out of the DVE/ACT
                                # FIFOs)
                                dlo = kt * 128
                                nc.gpsimd.tensor_mul(
                                    et[:, dlo:dlo + 128], et[:, dlo:dlo + 128],
                                    tri[:])

                        def emit_sum_av(seg, h=h):
                            kt, lo, hi, diag = seg
                            et = exp_tiles[kt]
                            last_kt = 3 if hi <= 512 else MT - 1
                            nc.tensor.matmul(
                                pssum[:, lo:hi], ones[:], et[:, lo:hi],
                                start=(kt == 0), stop=(kt == last_kt),
                            )
                            nc.tensor.matmul(
                                psout[:, lo:hi],
                                v_sb[:, kt * OPC + h * 128: kt * OPC + (h + 1) * 128],
                                et[:, lo:hi],
                                start=(kt == 0), stop=(kt == last_kt),
                            )

                        SKEW = 3
                        sl = seg_list()
                        for i, seg in enumerate(sl):
                            emit_scores(seg)
                            if i >= SKEW:
                                emit_sum_av(sl[i - SKEW])
                        for seg in sl[-SKEW:]:
                            emit_sum_av(seg)

                        # copy both psums to SBUF immediately (frees banks for
                        # the next head); recip+normalize run off-PSUM
                        sums_sb = sumsp.tile([128, SEQ], f32, tag="sums")
                        av_sb = sumsp.tile([128, SEQ], f32, tag="av")
                        nc.scalar.copy(sums_sb[:], pssum[:])
                        nc.vector.tensor_copy(av_sb[:], psout[:])
                        rec = recipp.tile([128, SEQ], f32, tag="recip")
                        nc.vector.reciprocal_approx_fast(rec[:], sums_sb[:])
                        nc.vector.tensor_mul(
                            attn[:, h * SEQ:(h + 1) * SEQ], av_sb[:], rec[:])

                # ---------- output projection (partial) ----------
                with (
                    tc.tile_pool(name="outs", bufs=2) as outsp,
                    tc.tile_pool(name="psW", bufs=8, space="PSUM") as psW,
                ):
                    for mt in range(MT):
                        outt = outsp.tile([128, DIM], bf16, tag="outs")
                        pss = [psW.tile([128, 512], f32, tag="psW", name=f"psW{nt}")
                               for nt in range(8)]
                        for ktw in range(HPC):
                            for nt in range(8):
                                nc.tensor.matmul(
                                    pss[nt][:],
                                    attn[:, ktw * SEQ + mt * 128: ktw * SEQ + (mt + 1) * 128],
                                    wo_sb[:, ktw * DIM + nt * 512: ktw * DIM + (nt + 1) * 512],
                                    start=(ktw == 0), stop=(ktw == HPC - 1),
                                )
                        for nt in range(8):
                            if nt % 2 == 0:
                                nc.scalar.copy(outt[:, nt * 512:(nt + 1) * 512], pss[nt][:])
                            else:
                                nc.vector.tensor_copy(outt[:, nt * 512:(nt + 1) * 512], pss[nt][:])
                        for q in range(8):
                            eng = nc.sync if q % 2 == 0 else nc.gpsimd
                            eng.dma_start(
                                out_d[mt * 128:(mt + 1) * 128, q * 512:(q + 1) * 512],
                                outt[:, q * 512:(q + 1) * 512])

    nc.compile()
    return nc


def _get_nc():
    if "nc" not in _STATE:
        _STATE["nc"] = _build_nc()
    return _STATE["nc"]


def _part_major(m):
    """[KT*128, F] -> [128, KT*F]: partition-major repack so each SBUF
    partition's data is one contiguous DRAM run."""
    kt = m.shape[0] // 128
    return np.ascontiguousarray(
        m.reshape(kt, 128, -1).transpose(1, 0, 2)).reshape(128, -1)


def _prep_inputs(x, wq_w, wq_s, wk_w, wk_s, wv_w, wv_s, wo_w, wo_s, cos, sin):
    """Host-side shard prep. Returns in_maps (list of 8 dicts)."""
    xt = _part_major(np.asarray(x, np.float32)[0].T.astype(F16))

    dq = _dequant(wq_w, wq_s).T   # [b, a] = lhsT full
    dk = _dequant(wk_w, wk_s).T
    dv = _dequant(wv_w, wv_s).T
    do = _dequant(wo_w, wo_s).T

    cos = np.asarray(cos, np.float32)
    sin = np.asarray(sin, np.float32)
    cfull = np.concatenate([cos, cos], axis=1).T          # [128, SEQ]
    ssign = np.concatenate([-sin, sin], axis=1).T         # [128, SEQ]
    alpha = 1.0 / np.sqrt(np.float32(HD))
    cq = (cfull * alpha).astype(F16)
    sq = (ssign * alpha).astype(F16)
    ck = cfull.astype(F16)
    sk = ssign.astype(F16)
    tri = (np.arange(128)[:, None] <= np.arange(128)[None, :]).astype(BF16)
    one = np.ones((128, 128), BF16)

    in_maps = []
    for c in range(NCORES):
        sl = slice(c * OPC, (c + 1) * OPC)
        # strip: [4096, 512] -> [ot, c128, kt*128+o]
        def strip(m):
            t = m[:, sl].astype(F16).reshape(KT, 128, HPC, 128)
            return np.ascontiguousarray(t.transpose(2, 1, 0, 3)).reshape(
                HPC, 128, KT * 128)
        # slab: [4096, 512] -> [g8, c128, ot*512 + ki*128 + o]
        def slab(m):
            t = m[:, sl].astype(F16).reshape(8, 4, 128, HPC, 128)
            return np.ascontiguousarray(t.transpose(0, 2, 3, 1, 4)).reshape(
                8, 128, HPC * 512)
        in_maps.append({
            "xt": xt,
            "wq": slab(dq),
            "wk": strip(dk),
            "wv": np.ascontiguousarray(dv[:, sl].astype(F16)).reshape(KT, 128, OPC),
            "wo": _part_major(do[sl, :].astype(BF16)),
            "cq": cq, "sq": sq, "ck": ck, "sk": sk, "tri": tri, "one": one,
        })
    return in_maps


def kernel(x, wq_w, wq_s, wk_w, wk_s, wv_w, wv_s, wo_w, wo_s,
           cos, sin, cache_k, cache_v, mask, start_pos, _trace=False):
    from concourse.bass_utils import run_bass_kernel_spmd

    nc = _get_nc()
    in_maps = _prep_inputs(x, wq_w, wq_s, wk_w, wk_s, wv_w, wv_s,
                           wo_w, wo_s, cos, sin)
    res = run_bass_kernel_spmd(nc, in_maps, list(range(NCORES)), trace=_trace)
    _STATE["last_result"] = res
    out = np.zeros((SEQ, DIM), np.float64)
    for c in range(NCORES):
        out += res.results[c]["out"].astype(np.float64)
    return out.astype(np.float32).reshape(1, SEQ, DIM)
